# revision 30
# baseline (speedup 1.0000x reference)
"""Trainium2 Bass kernel for nn_CNN_ternary (ternary CNN, 8-core data parallel).

Strategy:
  - All weights/BN folded on host (fp64). Activations after every layer are
    2x ternary {-2,0,2} (the 0.5 factor is folded into the next layer's
    weights, keeping every conv/fc psum integer-exact).
  - L1 (continuous input): x split into 3 stacked fp16 planes (hi/lo1/lo2),
    single K=108 matmul per output tile reproduces fp32-accurate conv.
  - Ternarization via ScalarE Sign with per-channel threshold bias APs:
      tern2(p) = Sign(p - thp) + Sign(p - thm)   in {-2, 0, 2}
    Integer layers: thresholds at half-integers (margin 0.5, exact).
    L1: margin-guarded fp32 thresholds (host-nudged lo2 plane).
  - Pools run on PSUM before ternarize (max commutes with monotone affine).
  - L3 matmuls col-split via tile_position for 4-way PE concurrency.
"""

import sys

sys.path.insert(0, "/opt/trn_rl_repo")

import numpy as np
import ml_dtypes

DELTA = 0.1
BN_EPS = 1e-5
N_CORES = 8
B_FULL = 4096
BC = B_FULL // N_CORES  # 512 per core
BT = 64                 # batch tile
NBT = BC // BT          # 8
NSTRIP = 4
NBQ = BT // NSTRIP      # 16


def _tern(t):
    return np.where(np.abs(t) < DELTA, 0.0, np.sign(t))


def _bf16(x):
    return np.asarray(x, dtype=np.float32).astype(ml_dtypes.bfloat16)


def _affine(i, d):
    """Return (A, B) fp64 such that z_i = A*conv + B, conv using tern weights."""
    g = d[f"g{i}"].astype(np.float64)
    be = d[f"be{i}"].astype(np.float64)
    m = d[f"m{i}"].astype(np.float64)
    v = d[f"v{i}"].astype(np.float64)
    tb = _tern(d[f"b{i}"].astype(np.float64))
    A = g / np.sqrt(v + BN_EPS)
    B = (tb - m) * A + be
    return A, B


def _int_thresholds(A, B, pmax):
    """Half-integer thresholds for integer psum p: tern=+1 iff p>=Kp,
    -1 iff p<=Km (after sg fold so effective A>0). Host-verified over range."""
    sg = np.where(A < 0, -1.0, 1.0)
    Aa = np.abs(A)
    thp = (DELTA - B) / Aa
    thm = (-DELTA - B) / Aa
    Kp = np.ceil(thp)
    Kp = np.where(Kp < thp, Kp + 1, Kp)
    Km = np.floor(thm)
    Km = np.where(Km > thm, Km - 1, Km)
    assert np.all(Kp - Km >= 1)
    p = np.arange(-pmax, pmax + 1, dtype=np.float64)
    z = Aa[:, None] * p[None, :] + (B * sg ** 0 if False else B)[:, None]
    # true ternary decision in fp64 z-space (A>0 after fold)
    true_t = np.where(z >= DELTA, 1, np.where(z <= -DELTA, -1, 0))
    pred_t = (p[None, :] >= Kp[:, None]).astype(np.int64) - (
        p[None, :] <= Km[:, None]).astype(np.int64)
    assert np.array_equal(true_t, pred_t), "threshold verification failed"
    thp_half = (Kp - 0.5).astype(np.float32)
    thm_half = (Km + 0.5).astype(np.float32)
    return sg, thp_half, thm_half


def _build_host_tensors(inputs):
    d = inputs
    A1, B1 = _affine(1, d)
    A2, B2 = _affine(2, d)
    A3, B3 = _affine(3, d)
    A4, B4 = _affine(4, d)

    sg1 = np.where(A1 < 0, -1.0, 1.0)
    thp1 = ((DELTA - B1) / np.abs(A1)).astype(np.float32)   # [32]
    thm1 = ((-DELTA - B1) / np.abs(A1)).astype(np.float32)

    sg2, thp2, thm2 = _int_thresholds(A2, B2, pmax=150)     # [64]
    sg3, thp3, thm3 = _int_thresholds(A3, B3, pmax=300)     # [128]
    sg4, thp4, thm4 = _int_thresholds(A4, B4, pmax=900)     # [128]

    w1t = _tern(d["w1"].astype(np.float64))[:, 0, 0, :] * sg1[:, None]   # [32,9]
    # next-layer weights carry 0.5 (input is 2x ternary) and sg fold
    w2t = _tern(d["w2"].astype(np.float64)) * (0.5 * sg2)[:, None, None, None]
    w3t = _tern(d["w3"].astype(np.float64)) * (0.5 * sg3)[:, None, None, None]
    w4t = _tern(d["w4"].astype(np.float64)) * (0.5 * sg4)[:, None, None, None]
    wft = _tern(d["wf"].astype(np.float64)) * 0.5                        # [10,2048]
    tbf = _tern(d["bf"].astype(np.float64)).astype(np.float32)           # [10]

    # --- lhsT tensors ---
    T1 = np.zeros((108, 128), np.float64)
    for bh in range(4):
        for t in range(9):
            for s in range(3):
                T1[s * 36 + bh * 9 + t, bh * 32:bh * 32 + 32] = w1t[:, t]
    T1 = T1.astype(np.float16)

    W2 = np.zeros((128, 3, 64), np.float64)
    for g in range(4):
        W2[32 * g:32 * g + 32] = w2t[:, :, 0, :].transpose(1, 2, 0)
    W2 = _bf16(W2)

    W3 = np.zeros((128, 3, 128), np.float64)
    for ch in range(2):
        W3[64 * ch:64 * ch + 64] = w3t[:, :, 0, :].transpose(1, 2, 0)
    W3 = _bf16(W3)

    W4 = _bf16(w4t[:, :, :, 0].transpose(1, 2, 0))  # [ci,h,c]
    WF = _bf16(wft.reshape(10, 128, 16).transpose(1, 2, 0))

    # --- threshold bias vectors [128,1] fp32 (bias = -th) ---
    vec = {}
    vec["B1P"] = np.tile(-thp1, 4)[:, None].astype(np.float32)
    vec["B1M"] = np.tile(-thm1, 4)[:, None].astype(np.float32)
    vec["B2P"] = np.tile(-thp2, 2)[:, None].astype(np.float32)
    vec["B2M"] = np.tile(-thm2, 2)[:, None].astype(np.float32)
    vec["N2M"] = np.tile(thm2 - 0.5, 2)[:, None].astype(np.float32)  # = Km
    vec["N1M"] = np.tile(thm1, 4)[:, None].astype(np.float32)
    vec["N3M"] = (thm3 - 0.5)[:, None].astype(np.float32)            # = Km3
    vec["B3P"] = (-thp3)[:, None].astype(np.float32)
    vec["B3M"] = (-thm3)[:, None].astype(np.float32)
    vec["B4P"] = (-thp4)[:, None].astype(np.float32)
    vec["B4M"] = (-thm4)[:, None].astype(np.float32)
    TBF = np.zeros((16, 1), np.float32)
    TBF[:10, 0] = tbf

    consts = dict(T1=T1, W2=W2, W3=W3, W4=W4, WF=WF, TBF=TBF, **vec)

    # --- X1 im2col per core: [108, NBT*NBQ*6*64] fp16 ---
    x = d["x"].astype(np.float32)[:, 0]          # [4096, 6, 128]
    xp = np.pad(x, ((0, 0), (0, 0), (4, 4)))     # [4096, 6, 136]
    x0 = xp.astype(np.float16)
    r1 = (xp - x0.astype(np.float32))
    x1 = r1.astype(np.float16)
    x2f = (r1 - x1.astype(np.float32))           # fp32 working copy of lo2

    j = np.arange(64)
    t = np.arange(9)
    qidx = (2 * j[None, :] + t[:, None])         # [9, 64]

    # --- margin guard (as baseline): nudge lo2 away from thresholds ---
    thp1g = (DELTA - B1) / np.abs(A1)
    thm1g = (-DELTA - B1) / np.abs(A1)
    TOL = 1.5e-5
    w64 = w1t.astype(np.float64)
    xs64 = (x0.astype(np.float64) + x1.astype(np.float64) + x2f.astype(np.float64))
    for _pass in range(3):
        nfix = 0
        for b0 in range(0, B_FULL, 512):
            blk = xs64[b0:b0 + 512]
            pe = np.einsum('bhtj,ct->bchj', blk[:, :, qidx], w64)
            for thr in (thp1g, thm1g):
                dist = pe - thr[None, :, None, None]
                bad = np.argwhere(np.abs(dist) < TOL)
                for bb, cc, hh, jj in bad:
                    dv = dist[bb, cc, hh, jj]
                    dp = np.sign(dv) * (2.0 * TOL - abs(dv)) if dv != 0 else 2.0 * TOL
                    for tt in range(9):
                        q = 2 * jj + tt
                        if w64[cc, tt] != 0 and 4 <= q < 132:
                            x2f[b0 + bb, hh, q] += np.float32(dp / w64[cc, tt])
                            xs64[b0 + bb, hh, q] = (x0[b0 + bb, hh, q].astype(np.float64)
                                                    + x1[b0 + bb, hh, q].astype(np.float64)
                                                    + np.float64(np.float16(x2f[b0 + bb, hh, q])))
                            nfix += 1
                            break
        if nfix == 0:
            break
    x2 = x2f.astype(np.float16)
    splits = [x0, x1, x2]

    X1s = []
    for cr in range(N_CORES):
        X1 = np.empty((108, NBT * NBQ * 6 * 64), np.float16)
        for s in range(3):
            xs = splits[s]
            for bh in range(4):
                bidx = (cr * BC + np.arange(NBT)[:, None] * BT + bh * NBQ
                        + np.arange(NBQ)[None, :]).reshape(-1)
                blk = xs[bidx][:, :, qidx]
                blk = blk.transpose(2, 0, 1, 3)
                X1[s * 36 + bh * 9: s * 36 + bh * 9 + 9] = blk.reshape(9, -1)
        X1s.append(X1)
    return consts, X1s


def _build_program():
    import concourse.bass as bass
    import concourse.tile as tile
    from concourse import bacc, mybir

    F = mybir.dt.float32
    H = mybir.dt.float16
    BF = mybir.dt.bfloat16
    AO = mybir.AluOpType
    AF = mybir.ActivationFunctionType

    nc = bacc.Bacc("TRN2", target_bir_lowering=False)

    NCOL1 = NBT * NBQ * 6 * 64
    X1 = nc.dram_tensor("X1", [108, NCOL1], H, kind="ExternalInput")
    T1 = nc.dram_tensor("T1", [108, 128], H, kind="ExternalInput")
    W2 = nc.dram_tensor("W2", [128, 3, 64], BF, kind="ExternalInput")
    W3 = nc.dram_tensor("W3", [128, 3, 128], BF, kind="ExternalInput")
    W4 = nc.dram_tensor("W4", [128, 6, 128], BF, kind="ExternalInput")
    WF = nc.dram_tensor("WF", [128, 16, 10], BF, kind="ExternalInput")
    vecs = {}
    for nm in ["B1P", "B1M", "B2P", "B2M", "N2M", "N1M", "N3M",
               "B3P", "B3M", "B4P", "B4M"]:
        vecs[nm] = nc.dram_tensor(nm, [128, 1], F, kind="ExternalInput")
    TBF = nc.dram_tensor("TBF", [16, 1], F, kind="ExternalInput")
    OUT = nc.dram_tensor("OUT", [BC, 10], F, kind="ExternalOutput")

    from contextlib import ExitStack
    with tile.TileContext(nc) as tc, ExitStack() as es:
        wp = es.enter_context(tc.tile_pool(name="wp", bufs=1))
        xp_ = es.enter_context(tc.tile_pool(name="xp", bufs=2))
        ap1 = es.enter_context(tc.tile_pool(name="ap1", bufs=2))
        ap2 = es.enter_context(tc.tile_pool(name="ap2", bufs=2))
        ap3 = es.enter_context(tc.tile_pool(name="ap3", bufs=2))
        ap4 = es.enter_context(tc.tile_pool(name="ap4", bufs=2))
        stg = es.enter_context(tc.tile_pool(name="stg", bufs=3))
        ps1p = es.enter_context(tc.tile_pool(name="ps1p", bufs=4, space="PSUM"))
        psAp = es.enter_context(tc.tile_pool(name="psAp", bufs=2, space="PSUM"))
        ps4p = es.enter_context(tc.tile_pool(name="ps4p", bufs=1, space="PSUM"))

        t1t = wp.tile([108, 128], H)
        nc.sync.dma_start(t1t[:], T1[:])
        w2t = wp.tile([128, 3, 64], BF)
        nc.sync.dma_start(w2t[:], W2[:])
        w3t = wp.tile([128, 3, 128], BF)
        nc.sync.dma_start(w3t[:], W3[:])
        w4t = wp.tile([128, 6, 128], BF)
        nc.sync.dma_start(w4t[:], W4[:])
        wft = wp.tile([128, 16, 10], BF)
        nc.sync.dma_start(wft[:], WF[:])
        vt = {}
        for nm, dr in vecs.items():
            vt[nm] = wp.tile([128, 1], F, tag=nm, name=nm.lower())
            nc.sync.dma_start(vt[nm][:], dr[:])
        tbft = wp.tile([16, 1], F)
        nc.sync.dma_start(tbft[:], TBF[:])

        for bt in range(NBT):
            # ---------- L1 ----------
            x1t = xp_.tile([108, NBQ * 6 * 64], H, tag="x1")
            nc.sync.dma_start(
                x1t[:], X1[:, bt * NBQ * 6 * 64:(bt + 1) * NBQ * 6 * 64])
            a1 = ap1.tile([128, NBQ, 6, 34], BF, tag="a1")
            nc.vector.memset(a1[:, :, :, 0:1], 0.0)
            nc.vector.memset(a1[:, :, :, 33:34], 0.0)
            # bq order matches L2's (ck) consumption: ck reads bq {2ck,2ck+1,
            # 8+2ck,8+2ck+1}, so finish those quartets first.
            bq_order = [b for ck in range(4)
                        for b in (2 * ck, 2 * ck + 1, 8 + 2 * ck, 9 + 2 * ck)]
            for bq in bq_order:
                ps = ps1p.tile([128, 384], F, tag="ps1")
                nc.tensor.matmul(ps[:], t1t[:],
                                 x1t[:, bq * 384:(bq + 1) * 384],
                                 start=True, stop=True)
                sv = ps[:].rearrange("p (h v e) -> p h v e", v=32, e=2)
                pl = stg.tile([128, 6, 32], F, tag="l1pl")
                nc.vector.tensor_reduce(pl[:], sv[:], mybir.AxisListType.X,
                                        AO.max)
                sE = stg.tile([128, 6, 32], BF, tag="l1se")
                nc.scalar.activation(sE[:], pl[:], AF.Sign, bias=vt["B1P"][:])
                # gpsimd: Sign(pl-thm) == 1 - 2*[pl<=thm]  (margin-guarded)
                sO = stg.tile([128, 6, 32], F, tag="l1so")
                nc.gpsimd.tensor_scalar(sO[:], pl[:], vt["N1M"][:], -2.0,
                                        AO.is_le, AO.mult)
                nc.gpsimd.tensor_scalar(sO[:], sO[:], 1.0, None, AO.add)
                nc.vector.tensor_tensor(a1[:, bq, :, 1:33], sE[:], sO[:],
                                        AO.add)
            # ---------- L2 ----------
            a2 = ap2.tile([128, 4, 8, 6, 34], BF, tag="a2")
            nc.vector.memset(a2[:, :, :, :, 0:1], 0.0)
            nc.vector.memset(a2[:, :, :, :, 33:34], 0.0)
            for ck in range(4):
                for g in range(4):
                    ps2 = psAp.tile([128, 384], F, tag="psA", name="ps2")
                    for t in range(3):
                        for ch in range(2):
                            bq0 = ch * 8 + ck * 2
                            nc.tensor.matmul(
                                ps2[64 * ch:64 * ch + 64, :],
                                w2t[32 * g:32 * g + 32, t, :],
                                a1[32 * g:32 * g + 32, bq0:bq0 + 2, :, t:t + 32],
                                start=(t == 0), stop=(t == 2),
                                tile_position=(32 * g, 64 * ch))
                    sE2 = stg.tile([128, 384], BF, tag="l2se")
                    nc.scalar.activation(sE2[:], ps2[:], AF.Sign,
                                         bias=vt["B2P"][:])
                    sO2 = stg.tile([128, 384], BF, tag="l2so")
                    nc.scalar.activation(sO2[:], ps2[:], AF.Sign,
                                         bias=vt["B2M"][:])
                    nc.vector.tensor_tensor(
                        a2[:, g, ck * 2:ck * 2 + 2, :, 1:33],
                        sE2[:].rearrange("p (b h v) -> p b h v", b=2, h=6),
                        sO2[:].rearrange("p (b h v) -> p b h v", b=2, h=6),
                        AO.add)
            # ---------- L3 (col-split for PE concurrency) ----------
            a3 = ap3.tile([128, 4, 16, 6, 16], BF, tag="a3")
            for g in range(4):
                for bp in range(4):
                    for ch in range(2):
                        ps3 = psAp.tile([128, 384], F, tag="psA", name="ps3")
                        for t in range(3):
                            for hf in range(2):
                                cp = 64 * hf
                                nc.tensor.matmul(
                                    ps3[cp:cp + 64, :],
                                    w3t[64 * ch:64 * ch + 64, t, cp:cp + 64],
                                    a2[64 * ch:64 * ch + 64, g,
                                       bp * 2:bp * 2 + 2, :, t:t + 32],
                                    start=(t == 0), stop=(t == 2),
                                    tile_position=(64 * ch, cp))
                        pv = ps3[:].rearrange("p (b h v e) -> p b h v e",
                                              b=2, h=6, e=2)
                        pl3 = stg.tile([128, 2, 6, 16], F, tag="l3pl")
                        nc.vector.tensor_reduce(pl3[:], pv[:],
                                                mybir.AxisListType.X, AO.max)
                        sE3 = stg.tile([128, 2, 6, 16], BF, tag="l3se")
                        nc.scalar.activation(sE3[:], pl3[:], AF.Sign,
                                             bias=vt["B3P"][:])
                        sO3 = stg.tile([128, 2, 6, 16], F, tag="l3so")
                        nc.gpsimd.tensor_scalar(sO3[:], pl3[:], vt["N3M"][:],
                                                -2.0, AO.is_le, AO.mult)
                        nc.gpsimd.tensor_scalar(sO3[:], sO3[:], 1.0, None,
                                                AO.add)
                        nc.vector.tensor_tensor(
                            a3[:, g, ch * 8 + bp * 2:ch * 8 + bp * 2 + 2, :, :],
                            sE3[:], sO3[:], AO.add)
            # ---------- L4 ----------
            a4 = ap4.tile([128, 4, 16, 16], BF, tag="a4")
            for ck in range(2):
                ps4 = ps4p.tile([128, 512], F, tag="ps4")
                for h in range(6):
                    nc.tensor.matmul(ps4[:], w4t[:, h, :],
                                     a3[:, ck * 2:ck * 2 + 2, :, h, :],
                                     start=(h == 0), stop=(h == 5))
                sE4 = stg.tile([128, 512], BF, tag="l4se")
                nc.scalar.activation(sE4[:], ps4[:], AF.Sign,
                                     bias=vt["B4P"][:])
                sO4 = stg.tile([128, 512], BF, tag="l4so")
                nc.scalar.activation(sO4[:], ps4[:], AF.Sign,
                                     bias=vt["B4M"][:])
                nc.vector.tensor_tensor(
                    a4[:, ck * 2:ck * 2 + 2, :, :],
                    sE4[:].rearrange("p (g b v) -> p g b v", g=2, b=16),
                    sO4[:].rearrange("p (g b v) -> p g b v", g=2, b=16),
                    AO.add)
            # ---------- FC ----------
            psf = ps4p.tile([16, 64], F, tag="psf")
            for w in range(16):
                nc.tensor.matmul(psf[0:10, :], wft[:, w, :], a4[:, :, :, w],
                                 start=(w == 0), stop=(w == 15))
            fo = stg.tile([16, 64], F, tag="fo")
            nc.scalar.activation(fo[0:10, :], psf[0:10, :], AF.Identity,
                                 bias=tbft[0:10, :], scale=1.0)
            nc.sync.dma_start(
                OUT[bt * BT:(bt + 1) * BT, :].rearrange("b o -> o b"),
                fo[0:10, :])

    nc.finalize()
    return nc


_CACHED = {}


def kernel(**inputs):
    from concourse.bass_utils import run_bass_kernel_spmd

    consts, X1s = _build_host_tensors(inputs)
    if "nc" not in _CACHED:
        _CACHED["nc"] = _build_program()
    nc = _CACHED["nc"]

    in_maps = []
    for cr in range(N_CORES):
        m = {k: np.ascontiguousarray(v) for k, v in consts.items()}
        m["X1"] = np.ascontiguousarray(X1s[cr])
        in_maps.append(m)

    res = run_bass_kernel_spmd(nc, in_maps, list(range(N_CORES)))
    out = np.concatenate([res.results[cr]["OUT"] for cr in range(N_CORES)], 0)
    return out.astype(np.float32)


# revision 34
# speedup vs baseline: 4.0803x; 4.0803x over previous
"""Trainium2 Bass kernel for nn_CNN_ternary (ternary CNN, 8-core data parallel).

Strategy:
  - All weights/BN folded on host (fp64). Activations after every layer are
    2x ternary {-2,0,2} (the 0.5 factor is folded into the next layer's
    weights, keeping every conv/fc psum integer-exact).
  - L1 (continuous input): x split into 3 stacked fp16 planes (hi/lo1/lo2),
    single K=108 matmul per output tile reproduces fp32-accurate conv.
  - Ternarization via ScalarE Sign with per-channel threshold bias APs:
      tern2(p) = Sign(p - thp) + Sign(p - thm)   in {-2, 0, 2}
    Integer layers: thresholds at half-integers (margin 0.5, exact).
    L1: margin-guarded fp32 thresholds (host-nudged lo2 plane).
  - Pools run on PSUM before ternarize (max commutes with monotone affine).
  - L3 matmuls col-split via tile_position for 4-way PE concurrency.
"""

import sys

sys.path.insert(0, "/opt/trn_rl_repo")

import numpy as np
import ml_dtypes

DELTA = 0.1
BN_EPS = 1e-5
N_CORES = 8
B_FULL = 4096
BC = B_FULL // N_CORES  # 512 per core
BT = 64                 # batch tile
NBT = BC // BT          # 8
NSTRIP = 4
NBQ = BT // NSTRIP      # 16


def _tern(t):
    return np.where(np.abs(t) < DELTA, 0.0, np.sign(t))


def _bf16(x):
    return np.asarray(x, dtype=np.float32).astype(ml_dtypes.bfloat16)


def _affine(i, d):
    """Return (A, B) fp64 such that z_i = A*conv + B, conv using tern weights."""
    g = d[f"g{i}"].astype(np.float64)
    be = d[f"be{i}"].astype(np.float64)
    m = d[f"m{i}"].astype(np.float64)
    v = d[f"v{i}"].astype(np.float64)
    tb = _tern(d[f"b{i}"].astype(np.float64))
    A = g / np.sqrt(v + BN_EPS)
    B = (tb - m) * A + be
    return A, B


def _int_thresholds(A, B, pmax):
    """Half-integer thresholds for integer psum p: tern=+1 iff p>=Kp,
    -1 iff p<=Km (after sg fold so effective A>0). Host-verified over range."""
    sg = np.where(A < 0, -1.0, 1.0)
    Aa = np.abs(A)
    thp = (DELTA - B) / Aa
    thm = (-DELTA - B) / Aa
    Kp = np.ceil(thp)
    Kp = np.where(Kp < thp, Kp + 1, Kp)
    Km = np.floor(thm)
    Km = np.where(Km > thm, Km - 1, Km)
    assert np.all(Kp - Km >= 1)
    p = np.arange(-pmax, pmax + 1, dtype=np.float64)
    z = Aa[:, None] * p[None, :] + (B * sg ** 0 if False else B)[:, None]
    # true ternary decision in fp64 z-space (A>0 after fold)
    true_t = np.where(z >= DELTA, 1, np.where(z <= -DELTA, -1, 0))
    pred_t = (p[None, :] >= Kp[:, None]).astype(np.int64) - (
        p[None, :] <= Km[:, None]).astype(np.int64)
    assert np.array_equal(true_t, pred_t), "threshold verification failed"
    thp_half = (Kp - 0.5).astype(np.float32)
    thm_half = (Km + 0.5).astype(np.float32)
    return sg, thp_half, thm_half


def _build_host_tensors(inputs):
    d = inputs
    A1, B1 = _affine(1, d)
    A2, B2 = _affine(2, d)
    A3, B3 = _affine(3, d)
    A4, B4 = _affine(4, d)

    sg1 = np.where(A1 < 0, -1.0, 1.0)
    thp1 = ((DELTA - B1) / np.abs(A1)).astype(np.float32)   # [32]
    thm1 = ((-DELTA - B1) / np.abs(A1)).astype(np.float32)

    sg2, thp2, thm2 = _int_thresholds(A2, B2, pmax=150)     # [64]
    sg3, thp3, thm3 = _int_thresholds(A3, B3, pmax=300)     # [128]
    sg4, thp4, thm4 = _int_thresholds(A4, B4, pmax=900)     # [128]

    w1t = _tern(d["w1"].astype(np.float64))[:, 0, 0, :] * sg1[:, None]   # [32,9]
    # next-layer weights carry 0.5 (input is 2x ternary) and sg fold
    w2t = _tern(d["w2"].astype(np.float64)) * (0.5 * sg2)[:, None, None, None]
    w3t = _tern(d["w3"].astype(np.float64)) * (0.5 * sg3)[:, None, None, None]
    w4t = _tern(d["w4"].astype(np.float64)) * (0.5 * sg4)[:, None, None, None]
    wft = _tern(d["wf"].astype(np.float64)) * 0.5                        # [10,2048]
    tbf = _tern(d["bf"].astype(np.float64)).astype(np.float32)           # [10]

    # --- lhsT tensors ---
    T1 = np.zeros((108, 128), np.float64)
    for bh in range(4):
        for t in range(9):
            for s in range(3):
                T1[s * 36 + bh * 9 + t, bh * 32:bh * 32 + 32] = w1t[:, t]
    T1 = T1.astype(np.float16)

    W2 = np.zeros((128, 3, 64), np.float64)
    for g in range(4):
        W2[32 * g:32 * g + 32] = w2t[:, :, 0, :].transpose(1, 2, 0)
    W2 = _bf16(W2)

    W3 = np.zeros((128, 3, 128), np.float64)
    for ch in range(2):
        W3[64 * ch:64 * ch + 64] = w3t[:, :, 0, :].transpose(1, 2, 0)
    W3 = _bf16(W3)

    W4 = _bf16(w4t[:, :, :, 0].transpose(1, 2, 0))  # [ci,h,c]
    WF = _bf16(wft.reshape(10, 128, 16).transpose(1, 2, 0))

    # --- threshold bias vectors [128,1] fp32 (bias = -th) ---
    vec = {}
    vec["B1P"] = np.tile(-thp1, 4)[:, None].astype(np.float32)
    vec["B1M"] = np.tile(-thm1, 4)[:, None].astype(np.float32)
    vec["B2P"] = np.tile(-thp2, 2)[:, None].astype(np.float32)
    vec["B2M"] = np.tile(-thm2, 2)[:, None].astype(np.float32)
    vec["N2M"] = np.tile(thm2 - 0.5, 2)[:, None].astype(np.float32)  # = Km
    vec["N3M"] = (thm3 - 0.5)[:, None].astype(np.float32)            # = Km3
    vec["B3P"] = (-thp3)[:, None].astype(np.float32)
    vec["B3M"] = (-thm3)[:, None].astype(np.float32)
    vec["B4P"] = (-thp4)[:, None].astype(np.float32)
    vec["B4M"] = (-thm4)[:, None].astype(np.float32)
    TBF = np.zeros((16, 1), np.float32)
    TBF[:10, 0] = tbf

    consts = dict(T1=T1, W2=W2, W3=W3, W4=W4, WF=WF, TBF=TBF, **vec)

    # --- X1 im2col per core: [108, NBT*NBQ*6*64] fp16 ---
    x = d["x"].astype(np.float32)[:, 0]          # [4096, 6, 128]
    xp = np.pad(x, ((0, 0), (0, 0), (4, 4)))     # [4096, 6, 136]
    x0 = xp.astype(np.float16)
    r1 = (xp - x0.astype(np.float32))
    x1 = r1.astype(np.float16)
    x2f = (r1 - x1.astype(np.float32))           # fp32 working copy of lo2

    j = np.arange(64)
    t = np.arange(9)
    qidx = (2 * j[None, :] + t[:, None])         # [9, 64]

    # --- margin guard (as baseline): nudge lo2 away from thresholds ---
    thp1g = (DELTA - B1) / np.abs(A1)
    thm1g = (-DELTA - B1) / np.abs(A1)
    TOL = 1.5e-5
    w64 = w1t.astype(np.float64)
    xs64 = (x0.astype(np.float64) + x1.astype(np.float64) + x2f.astype(np.float64))
    for _pass in range(3):
        nfix = 0
        for b0 in range(0, B_FULL, 512):
            blk = xs64[b0:b0 + 512]
            pe = np.einsum('bhtj,ct->bchj', blk[:, :, qidx], w64)
            for thr in (thp1g, thm1g):
                dist = pe - thr[None, :, None, None]
                bad = np.argwhere(np.abs(dist) < TOL)
                for bb, cc, hh, jj in bad:
                    dv = dist[bb, cc, hh, jj]
                    dp = np.sign(dv) * (2.0 * TOL - abs(dv)) if dv != 0 else 2.0 * TOL
                    for tt in range(9):
                        q = 2 * jj + tt
                        if w64[cc, tt] != 0 and 4 <= q < 132:
                            x2f[b0 + bb, hh, q] += np.float32(dp / w64[cc, tt])
                            xs64[b0 + bb, hh, q] = (x0[b0 + bb, hh, q].astype(np.float64)
                                                    + x1[b0 + bb, hh, q].astype(np.float64)
                                                    + np.float64(np.float16(x2f[b0 + bb, hh, q])))
                            nfix += 1
                            break
        if nfix == 0:
            break
    x2 = x2f.astype(np.float16)
    splits = [x0, x1, x2]

    X1s = []
    for cr in range(N_CORES):
        X1 = np.empty((108, NBT * NBQ * 6 * 64), np.float16)
        for s in range(3):
            xs = splits[s]
            for bh in range(4):
                bidx = (cr * BC + np.arange(NBT)[:, None] * BT + bh * NBQ
                        + np.arange(NBQ)[None, :]).reshape(-1)
                blk = xs[bidx][:, :, qidx]
                blk = blk.transpose(2, 0, 1, 3)
                X1[s * 36 + bh * 9: s * 36 + bh * 9 + 9] = blk.reshape(9, -1)
        X1s.append(X1)
    return consts, X1s


def _build_program():
    import concourse.bass as bass
    import concourse.tile as tile
    from concourse import bacc, mybir

    F = mybir.dt.float32
    H = mybir.dt.float16
    BF = mybir.dt.bfloat16
    AO = mybir.AluOpType
    AF = mybir.ActivationFunctionType

    nc = bacc.Bacc("TRN2", target_bir_lowering=False)

    NCOL1 = NBT * NBQ * 6 * 64
    X1 = nc.dram_tensor("X1", [108, NCOL1], H, kind="ExternalInput")
    T1 = nc.dram_tensor("T1", [108, 128], H, kind="ExternalInput")
    W2 = nc.dram_tensor("W2", [128, 3, 64], BF, kind="ExternalInput")
    W3 = nc.dram_tensor("W3", [128, 3, 128], BF, kind="ExternalInput")
    W4 = nc.dram_tensor("W4", [128, 6, 128], BF, kind="ExternalInput")
    WF = nc.dram_tensor("WF", [128, 16, 10], BF, kind="ExternalInput")
    vecs = {}
    for nm in ["B1P", "B1M", "B2P", "B2M", "N2M", "N3M",
               "B3P", "B3M", "B4P", "B4M"]:
        vecs[nm] = nc.dram_tensor(nm, [128, 1], F, kind="ExternalInput")
    TBF = nc.dram_tensor("TBF", [16, 1], F, kind="ExternalInput")
    OUT = nc.dram_tensor("OUT", [BC, 10], F, kind="ExternalOutput")

    from contextlib import ExitStack
    with tile.TileContext(nc) as tc, ExitStack() as es:
        wp = es.enter_context(tc.tile_pool(name="wp", bufs=1))
        xp_ = es.enter_context(tc.tile_pool(name="xp", bufs=2))
        ap1 = es.enter_context(tc.tile_pool(name="ap1", bufs=2))
        ap2 = es.enter_context(tc.tile_pool(name="ap2", bufs=2))
        ap3 = es.enter_context(tc.tile_pool(name="ap3", bufs=2))
        ap4 = es.enter_context(tc.tile_pool(name="ap4", bufs=2))
        stg = es.enter_context(tc.tile_pool(name="stg", bufs=3))
        ps1p = es.enter_context(tc.tile_pool(name="ps1p", bufs=4, space="PSUM"))
        psAp = es.enter_context(tc.tile_pool(name="psAp", bufs=2, space="PSUM"))
        ps4p = es.enter_context(tc.tile_pool(name="ps4p", bufs=1, space="PSUM"))

        t1t = wp.tile([108, 128], H)
        nc.sync.dma_start(t1t[:], T1[:])
        w2t = wp.tile([128, 3, 64], BF)
        nc.sync.dma_start(w2t[:], W2[:])
        w3t = wp.tile([128, 3, 128], BF)
        nc.sync.dma_start(w3t[:], W3[:])
        w4t = wp.tile([128, 6, 128], BF)
        nc.sync.dma_start(w4t[:], W4[:])
        wft = wp.tile([128, 16, 10], BF)
        nc.sync.dma_start(wft[:], WF[:])
        vt = {}
        for nm, dr in vecs.items():
            vt[nm] = wp.tile([128, 1], F, tag=nm, name=nm.lower())
            nc.sync.dma_start(vt[nm][:], dr[:])
        tbft = wp.tile([16, 1], F)
        nc.sync.dma_start(tbft[:], TBF[:])

        for bt in range(NBT):
            # ---------- L1 ----------
            x1t = xp_.tile([108, NBQ * 6 * 64], H, tag="x1")
            nc.sync.dma_start(
                x1t[:], X1[:, bt * NBQ * 6 * 64:(bt + 1) * NBQ * 6 * 64])
            a1 = ap1.tile([128, NBQ, 6, 34], BF, tag="a1")
            nc.vector.memset(a1[:, :, :, 0:1], 0.0)
            nc.vector.memset(a1[:, :, :, 33:34], 0.0)
            # bq order matches L2's (ck) consumption: ck reads bq {2ck,2ck+1,
            # 8+2ck,8+2ck+1}, so finish those quartets first.
            bq_order = [b for ck in range(4)
                        for b in (2 * ck, 2 * ck + 1, 8 + 2 * ck, 9 + 2 * ck)]
            for bq in bq_order:
                ps = ps1p.tile([128, 384], F, tag="ps1")
                nc.tensor.matmul(ps[:], t1t[:],
                                 x1t[:, bq * 384:(bq + 1) * 384],
                                 start=True, stop=True)
                sv = ps[:].rearrange("p (h v e) -> p h v e", v=32, e=2)
                pl = stg.tile([128, 6, 32], F, tag="l1pl")
                nc.vector.tensor_reduce(pl[:], sv[:], mybir.AxisListType.X,
                                        AO.max)
                sE = stg.tile([128, 6, 32], BF, tag="l1se")
                nc.scalar.activation(sE[:], pl[:], AF.Sign, bias=vt["B1P"][:])
                sO = stg.tile([128, 6, 32], BF, tag="l1so")
                nc.scalar.activation(sO[:], pl[:], AF.Sign, bias=vt["B1M"][:])
                nc.vector.tensor_tensor(a1[:, bq, :, 1:33], sE[:], sO[:],
                                        AO.add)
            # ---------- L2 ----------
            a2 = ap2.tile([128, 4, 8, 6, 34], BF, tag="a2")
            nc.vector.memset(a2[:, :, :, :, 0:1], 0.0)
            nc.vector.memset(a2[:, :, :, :, 33:34], 0.0)
            for ck in range(4):
                for g in range(4):
                    ps2 = psAp.tile([128, 384], F, tag="psA", name="ps2")
                    for t in range(3):
                        for ch in range(2):
                            bq0 = ch * 8 + ck * 2
                            nc.tensor.matmul(
                                ps2[64 * ch:64 * ch + 64, :],
                                w2t[32 * g:32 * g + 32, t, :],
                                a1[32 * g:32 * g + 32, bq0:bq0 + 2, :, t:t + 32],
                                start=(t == 0), stop=(t == 2),
                                tile_position=(32 * g, 64 * ch))
                    sE2 = stg.tile([128, 384], BF, tag="l2se")
                    nc.scalar.activation(sE2[:], ps2[:], AF.Sign,
                                         bias=vt["B2P"][:])
                    sO2 = stg.tile([128, 384], BF, tag="l2so")
                    nc.scalar.activation(sO2[:], ps2[:], AF.Sign,
                                         bias=vt["B2M"][:])
                    nc.vector.tensor_tensor(
                        a2[:, g, ck * 2:ck * 2 + 2, :, 1:33],
                        sE2[:].rearrange("p (b h v) -> p b h v", b=2, h=6),
                        sO2[:].rearrange("p (b h v) -> p b h v", b=2, h=6),
                        AO.add)
            # ---------- L3 (col-split for PE concurrency) ----------
            a3 = ap3.tile([128, 4, 16, 6, 16], BF, tag="a3")
            for g in range(4):
                for bp in range(4):
                    for ch in range(2):
                        ps3 = psAp.tile([128, 384], F, tag="psA", name="ps3")
                        for t in range(3):
                            for hf in range(2):
                                cp = 64 * hf
                                nc.tensor.matmul(
                                    ps3[cp:cp + 64, :],
                                    w3t[64 * ch:64 * ch + 64, t, cp:cp + 64],
                                    a2[64 * ch:64 * ch + 64, g,
                                       bp * 2:bp * 2 + 2, :, t:t + 32],
                                    start=(t == 0), stop=(t == 2),
                                    tile_position=(64 * ch, cp))
                        pv = ps3[:].rearrange("p (b h v e) -> p b h v e",
                                              b=2, h=6, e=2)
                        pl3 = stg.tile([128, 2, 6, 16], F, tag="l3pl")
                        nc.vector.tensor_reduce(pl3[:], pv[:],
                                                mybir.AxisListType.X, AO.max)
                        sE3 = stg.tile([128, 2, 6, 16], BF, tag="l3se")
                        nc.scalar.activation(sE3[:], pl3[:], AF.Sign,
                                             bias=vt["B3P"][:])
                        sO3 = stg.tile([128, 2, 6, 16], BF, tag="l3so")
                        if (g + bp + ch) % 2 == 0:
                            nc.scalar.activation(sO3[:], pl3[:], AF.Sign,
                                                 bias=vt["B3M"][:])
                        else:
                            # DVE: Sign(pl-thm) == 1 - 2*[pl<=Km] (integer pl)
                            nc.vector.tensor_scalar(sO3[:], pl3[:],
                                                    vt["N3M"][:], -2.0,
                                                    AO.is_le, AO.mult)
                            nc.vector.tensor_scalar(sO3[:], sO3[:], 1.0, None,
                                                    AO.add)
                        nc.vector.tensor_tensor(
                            a3[:, g, ch * 8 + bp * 2:ch * 8 + bp * 2 + 2, :, :],
                            sE3[:], sO3[:], AO.add)
            # ---------- L4 ----------
            a4 = ap4.tile([128, 4, 16, 16], BF, tag="a4")
            for ck in range(2):
                ps4 = ps4p.tile([128, 512], F, tag="ps4")
                for h in range(6):
                    nc.tensor.matmul(ps4[:], w4t[:, h, :],
                                     a3[:, ck * 2:ck * 2 + 2, :, h, :],
                                     start=(h == 0), stop=(h == 5))
                sE4 = stg.tile([128, 512], BF, tag="l4se")
                nc.scalar.activation(sE4[:], ps4[:], AF.Sign,
                                     bias=vt["B4P"][:])
                sO4 = stg.tile([128, 512], BF, tag="l4so")
                nc.scalar.activation(sO4[:], ps4[:], AF.Sign,
                                     bias=vt["B4M"][:])
                nc.vector.tensor_tensor(
                    a4[:, ck * 2:ck * 2 + 2, :, :],
                    sE4[:].rearrange("p (g b v) -> p g b v", g=2, b=16),
                    sO4[:].rearrange("p (g b v) -> p g b v", g=2, b=16),
                    AO.add)
            # ---------- FC ----------
            psf = ps4p.tile([16, 64], F, tag="psf")
            for w in range(16):
                nc.tensor.matmul(psf[0:10, :], wft[:, w, :], a4[:, :, :, w],
                                 start=(w == 0), stop=(w == 15))
            fo = stg.tile([16, 64], F, tag="fo")
            nc.scalar.activation(fo[0:10, :], psf[0:10, :], AF.Identity,
                                 bias=tbft[0:10, :], scale=1.0)
            nc.sync.dma_start(
                OUT[bt * BT:(bt + 1) * BT, :].rearrange("b o -> o b"),
                fo[0:10, :])

    nc.finalize()
    return nc


_CACHED = {}


def kernel(**inputs):
    from concourse.bass_utils import run_bass_kernel_spmd

    consts, X1s = _build_host_tensors(inputs)
    if "nc" not in _CACHED:
        _CACHED["nc"] = _build_program()
    nc = _CACHED["nc"]

    in_maps = []
    for cr in range(N_CORES):
        m = {k: np.ascontiguousarray(v) for k, v in consts.items()}
        m["X1"] = np.ascontiguousarray(X1s[cr])
        in_maps.append(m)

    res = run_bass_kernel_spmd(nc, in_maps, list(range(N_CORES)))
    out = np.concatenate([res.results[cr]["OUT"] for cr in range(N_CORES)], 0)
    return out.astype(np.float32)


# revision 35
# speedup vs baseline: 5.0102x; 1.2279x over previous
"""Trainium2 Bass kernel for nn_CNN_ternary (ternary CNN, 8-core data parallel).

Strategy:
  - All weights/BN folded on host (fp64). Activations after every layer are
    2x ternary {-2,0,2} (the 0.5 factor is folded into the next layer's
    weights, keeping every conv/fc psum integer-exact).
  - L1 (continuous input): x split into 3 stacked fp16 planes (hi/lo1/lo2),
    single K=108 matmul per output tile reproduces fp32-accurate conv.
  - Ternarization via ScalarE Sign with per-channel threshold bias APs:
      tern2(p) = Sign(p - thp) + Sign(p - thm)   in {-2, 0, 2}
    Integer layers: thresholds at half-integers (margin 0.5, exact).
    L1: margin-guarded fp32 thresholds (host-nudged lo2 plane).
  - Pools run on PSUM before ternarize (max commutes with monotone affine).
  - L3 matmuls col-split via tile_position for 4-way PE concurrency.
"""

import sys

sys.path.insert(0, "/opt/trn_rl_repo")

import numpy as np
import ml_dtypes

DELTA = 0.1
BN_EPS = 1e-5
N_CORES = 8
B_FULL = 4096
BC = B_FULL // N_CORES  # 512 per core
BT = 64                 # batch tile
NBT = BC // BT          # 8
NSTRIP = 4
NBQ = BT // NSTRIP      # 16


def _tern(t):
    return np.where(np.abs(t) < DELTA, 0.0, np.sign(t))


def _bf16(x):
    return np.asarray(x, dtype=np.float32).astype(ml_dtypes.bfloat16)


def _affine(i, d):
    """Return (A, B) fp64 such that z_i = A*conv + B, conv using tern weights."""
    g = d[f"g{i}"].astype(np.float64)
    be = d[f"be{i}"].astype(np.float64)
    m = d[f"m{i}"].astype(np.float64)
    v = d[f"v{i}"].astype(np.float64)
    tb = _tern(d[f"b{i}"].astype(np.float64))
    A = g / np.sqrt(v + BN_EPS)
    B = (tb - m) * A + be
    return A, B


def _int_thresholds(A, B, pmax):
    """Half-integer thresholds for integer psum p: tern=+1 iff p>=Kp,
    -1 iff p<=Km (after sg fold so effective A>0). Host-verified over range."""
    sg = np.where(A < 0, -1.0, 1.0)
    Aa = np.abs(A)
    thp = (DELTA - B) / Aa
    thm = (-DELTA - B) / Aa
    Kp = np.ceil(thp)
    Kp = np.where(Kp < thp, Kp + 1, Kp)
    Km = np.floor(thm)
    Km = np.where(Km > thm, Km - 1, Km)
    assert np.all(Kp - Km >= 1)
    p = np.arange(-pmax, pmax + 1, dtype=np.float64)
    z = Aa[:, None] * p[None, :] + (B * sg ** 0 if False else B)[:, None]
    # true ternary decision in fp64 z-space (A>0 after fold)
    true_t = np.where(z >= DELTA, 1, np.where(z <= -DELTA, -1, 0))
    pred_t = (p[None, :] >= Kp[:, None]).astype(np.int64) - (
        p[None, :] <= Km[:, None]).astype(np.int64)
    assert np.array_equal(true_t, pred_t), "threshold verification failed"
    thp_half = (Kp - 0.5).astype(np.float32)
    thm_half = (Km + 0.5).astype(np.float32)
    return sg, thp_half, thm_half


def _build_host_tensors(inputs):
    d = inputs
    A1, B1 = _affine(1, d)
    A2, B2 = _affine(2, d)
    A3, B3 = _affine(3, d)
    A4, B4 = _affine(4, d)

    sg1 = np.where(A1 < 0, -1.0, 1.0)
    thp1 = ((DELTA - B1) / np.abs(A1)).astype(np.float32)   # [32]
    thm1 = ((-DELTA - B1) / np.abs(A1)).astype(np.float32)

    sg2, thp2, thm2 = _int_thresholds(A2, B2, pmax=150)     # [64]
    sg3, thp3, thm3 = _int_thresholds(A3, B3, pmax=300)     # [128]
    sg4, thp4, thm4 = _int_thresholds(A4, B4, pmax=900)     # [128]

    w1t = _tern(d["w1"].astype(np.float64))[:, 0, 0, :] * sg1[:, None]   # [32,9]
    # next-layer weights carry 0.5 (input is 2x ternary) and sg fold
    w2t = _tern(d["w2"].astype(np.float64)) * (0.5 * sg2)[:, None, None, None]
    w3t = _tern(d["w3"].astype(np.float64)) * (0.5 * sg3)[:, None, None, None]
    w4t = _tern(d["w4"].astype(np.float64)) * (0.5 * sg4)[:, None, None, None]
    wft = _tern(d["wf"].astype(np.float64)) * 0.5                        # [10,2048]
    tbf = _tern(d["bf"].astype(np.float64)).astype(np.float32)           # [10]

    # --- lhsT tensors ---
    T1 = np.zeros((108, 128), np.float64)
    for bh in range(4):
        for t in range(9):
            for s in range(3):
                T1[s * 36 + bh * 9 + t, bh * 32:bh * 32 + 32] = w1t[:, t]
    T1 = T1.astype(np.float16)

    W2 = np.zeros((128, 3, 64), np.float64)
    for g in range(4):
        W2[32 * g:32 * g + 32] = w2t[:, :, 0, :].transpose(1, 2, 0)
    W2 = _bf16(W2)

    W3 = np.zeros((128, 3, 128), np.float64)
    for ch in range(2):
        W3[64 * ch:64 * ch + 64] = w3t[:, :, 0, :].transpose(1, 2, 0)
    W3 = _bf16(W3)

    W4 = _bf16(w4t[:, :, :, 0].transpose(1, 2, 0))  # [ci,h,c]
    WF = _bf16(wft.reshape(10, 128, 16).transpose(1, 2, 0))

    # --- threshold bias vectors [128,1] fp32 (bias = -th) ---
    vec = {}
    vec["B1P"] = np.tile(-thp1, 4)[:, None].astype(np.float32)
    vec["B1M"] = np.tile(-thm1, 4)[:, None].astype(np.float32)
    vec["B2P"] = np.tile(-thp2, 2)[:, None].astype(np.float32)
    vec["B2M"] = np.tile(-thm2, 2)[:, None].astype(np.float32)
    vec["N2M"] = np.tile(thm2 - 0.5, 2)[:, None].astype(np.float32)  # = Km
    vec["B3P"] = (-thp3)[:, None].astype(np.float32)
    vec["B3M"] = (-thm3)[:, None].astype(np.float32)
    vec["B4P"] = (-thp4)[:, None].astype(np.float32)
    vec["B4M"] = (-thm4)[:, None].astype(np.float32)
    TBF = np.zeros((16, 1), np.float32)
    TBF[:10, 0] = tbf

    consts = dict(T1=T1, W2=W2, W3=W3, W4=W4, WF=WF, TBF=TBF, **vec)

    # --- X1 im2col per core: [108, NBT*NBQ*6*64] fp16 ---
    x = d["x"].astype(np.float32)[:, 0]          # [4096, 6, 128]
    xp = np.pad(x, ((0, 0), (0, 0), (4, 4)))     # [4096, 6, 136]
    x0 = xp.astype(np.float16)
    r1 = (xp - x0.astype(np.float32))
    x1 = r1.astype(np.float16)
    x2f = (r1 - x1.astype(np.float32))           # fp32 working copy of lo2

    j = np.arange(64)
    t = np.arange(9)
    qidx = (2 * j[None, :] + t[:, None])         # [9, 64]

    # --- margin guard (as baseline): nudge lo2 away from thresholds ---
    thp1g = (DELTA - B1) / np.abs(A1)
    thm1g = (-DELTA - B1) / np.abs(A1)
    TOL = 1.5e-5
    w64 = w1t.astype(np.float64)
    xs64 = (x0.astype(np.float64) + x1.astype(np.float64) + x2f.astype(np.float64))
    for _pass in range(3):
        nfix = 0
        for b0 in range(0, B_FULL, 512):
            blk = xs64[b0:b0 + 512]
            pe = np.einsum('bhtj,ct->bchj', blk[:, :, qidx], w64)
            for thr in (thp1g, thm1g):
                dist = pe - thr[None, :, None, None]
                bad = np.argwhere(np.abs(dist) < TOL)
                for bb, cc, hh, jj in bad:
                    dv = dist[bb, cc, hh, jj]
                    dp = np.sign(dv) * (2.0 * TOL - abs(dv)) if dv != 0 else 2.0 * TOL
                    for tt in range(9):
                        q = 2 * jj + tt
                        if w64[cc, tt] != 0 and 4 <= q < 132:
                            x2f[b0 + bb, hh, q] += np.float32(dp / w64[cc, tt])
                            xs64[b0 + bb, hh, q] = (x0[b0 + bb, hh, q].astype(np.float64)
                                                    + x1[b0 + bb, hh, q].astype(np.float64)
                                                    + np.float64(np.float16(x2f[b0 + bb, hh, q])))
                            nfix += 1
                            break
        if nfix == 0:
            break
    x2 = x2f.astype(np.float16)
    splits = [x0, x1, x2]

    X1s = []
    for cr in range(N_CORES):
        X1 = np.empty((108, NBT * NBQ * 6 * 64), np.float16)
        for s in range(3):
            xs = splits[s]
            for bh in range(4):
                bidx = (cr * BC + np.arange(NBT)[:, None] * BT + bh * NBQ
                        + np.arange(NBQ)[None, :]).reshape(-1)
                blk = xs[bidx][:, :, qidx]
                blk = blk.transpose(2, 0, 1, 3)
                X1[s * 36 + bh * 9: s * 36 + bh * 9 + 9] = blk.reshape(9, -1)
        X1s.append(X1)
    return consts, X1s


def _build_program():
    import concourse.bass as bass
    import concourse.tile as tile
    from concourse import bacc, mybir

    F = mybir.dt.float32
    H = mybir.dt.float16
    BF = mybir.dt.bfloat16
    AO = mybir.AluOpType
    AF = mybir.ActivationFunctionType

    nc = bacc.Bacc("TRN2", target_bir_lowering=False)

    NCOL1 = NBT * NBQ * 6 * 64
    X1 = nc.dram_tensor("X1", [108, NCOL1], H, kind="ExternalInput")
    T1 = nc.dram_tensor("T1", [108, 128], H, kind="ExternalInput")
    W2 = nc.dram_tensor("W2", [128, 3, 64], BF, kind="ExternalInput")
    W3 = nc.dram_tensor("W3", [128, 3, 128], BF, kind="ExternalInput")
    W4 = nc.dram_tensor("W4", [128, 6, 128], BF, kind="ExternalInput")
    WF = nc.dram_tensor("WF", [128, 16, 10], BF, kind="ExternalInput")
    vecs = {}
    for nm in ["B1P", "B1M", "B2P", "B2M", "N2M", "B3P", "B3M", "B4P", "B4M"]:
        vecs[nm] = nc.dram_tensor(nm, [128, 1], F, kind="ExternalInput")
    TBF = nc.dram_tensor("TBF", [16, 1], F, kind="ExternalInput")
    OUT = nc.dram_tensor("OUT", [BC, 10], F, kind="ExternalOutput")

    from contextlib import ExitStack
    with tile.TileContext(nc) as tc, ExitStack() as es:
        wp = es.enter_context(tc.tile_pool(name="wp", bufs=1))
        xp_ = es.enter_context(tc.tile_pool(name="xp", bufs=2))
        ap1 = es.enter_context(tc.tile_pool(name="ap1", bufs=2))
        ap2 = es.enter_context(tc.tile_pool(name="ap2", bufs=2))
        ap3 = es.enter_context(tc.tile_pool(name="ap3", bufs=2))
        ap4 = es.enter_context(tc.tile_pool(name="ap4", bufs=2))
        stg = es.enter_context(tc.tile_pool(name="stg", bufs=3))
        ps1p = es.enter_context(tc.tile_pool(name="ps1p", bufs=4, space="PSUM"))
        psAp = es.enter_context(tc.tile_pool(name="psAp", bufs=2, space="PSUM"))
        ps4p = es.enter_context(tc.tile_pool(name="ps4p", bufs=1, space="PSUM"))

        t1t = wp.tile([108, 128], H)
        nc.sync.dma_start(t1t[:], T1[:])
        w2t = wp.tile([128, 3, 64], BF)
        nc.sync.dma_start(w2t[:], W2[:])
        w3t = wp.tile([128, 3, 128], BF)
        nc.sync.dma_start(w3t[:], W3[:])
        w4t = wp.tile([128, 6, 128], BF)
        nc.sync.dma_start(w4t[:], W4[:])
        wft = wp.tile([128, 16, 10], BF)
        nc.sync.dma_start(wft[:], WF[:])
        vt = {}
        for nm, dr in vecs.items():
            vt[nm] = wp.tile([128, 1], F, tag=nm, name=nm.lower())
            nc.sync.dma_start(vt[nm][:], dr[:])
        tbft = wp.tile([16, 1], F)
        nc.sync.dma_start(tbft[:], TBF[:])

        for bt in range(NBT):
            # ---------- L1 ----------
            x1t = xp_.tile([108, NBQ * 6 * 64], H, tag="x1")
            nc.sync.dma_start(
                x1t[:], X1[:, bt * NBQ * 6 * 64:(bt + 1) * NBQ * 6 * 64])
            a1 = ap1.tile([128, NBQ, 6, 34], BF, tag="a1")
            nc.vector.memset(a1[:, :, :, 0:1], 0.0)
            nc.vector.memset(a1[:, :, :, 33:34], 0.0)
            # bq order matches L2's (ck) consumption: ck reads bq {2ck,2ck+1,
            # 8+2ck,8+2ck+1}, so finish those quartets first.
            bq_order = [b for ck in range(4)
                        for b in (2 * ck, 2 * ck + 1, 8 + 2 * ck, 9 + 2 * ck)]
            for bq in bq_order:
                ps = ps1p.tile([128, 384], F, tag="ps1")
                nc.tensor.matmul(ps[:], t1t[:],
                                 x1t[:, bq * 384:(bq + 1) * 384],
                                 start=True, stop=True)
                sv = ps[:].rearrange("p (h v e) -> p h v e", v=32, e=2)
                pl = stg.tile([128, 6, 32], F, tag="l1pl")
                nc.vector.tensor_reduce(pl[:], sv[:], mybir.AxisListType.X,
                                        AO.max)
                sE = stg.tile([128, 6, 32], BF, tag="l1se")
                nc.scalar.activation(sE[:], pl[:], AF.Sign, bias=vt["B1P"][:])
                sO = stg.tile([128, 6, 32], BF, tag="l1so")
                nc.scalar.activation(sO[:], pl[:], AF.Sign, bias=vt["B1M"][:])
                nc.vector.tensor_tensor(a1[:, bq, :, 1:33], sE[:], sO[:],
                                        AO.add)
            # ---------- L2 ----------
            a2 = ap2.tile([128, 4, 8, 6, 34], BF, tag="a2")
            nc.vector.memset(a2[:, :, :, :, 0:1], 0.0)
            nc.vector.memset(a2[:, :, :, :, 33:34], 0.0)
            for ck in range(4):
                for g in range(4):
                    ps2 = psAp.tile([128, 384], F, tag="psA", name="ps2")
                    for t in range(3):
                        for ch in range(2):
                            bq0 = ch * 8 + ck * 2
                            nc.tensor.matmul(
                                ps2[64 * ch:64 * ch + 64, :],
                                w2t[32 * g:32 * g + 32, t, :],
                                a1[32 * g:32 * g + 32, bq0:bq0 + 2, :, t:t + 32],
                                start=(t == 0), stop=(t == 2),
                                tile_position=(32 * g, 64 * ch))
                    sE2 = stg.tile([128, 384], BF, tag="l2se")
                    nc.scalar.activation(sE2[:], ps2[:], AF.Sign,
                                         bias=vt["B2P"][:])
                    sO2 = stg.tile([128, 384], BF, tag="l2so")
                    nc.scalar.activation(sO2[:], ps2[:], AF.Sign,
                                         bias=vt["B2M"][:])
                    nc.vector.tensor_tensor(
                        a2[:, g, ck * 2:ck * 2 + 2, :, 1:33],
                        sE2[:].rearrange("p (b h v) -> p b h v", b=2, h=6),
                        sO2[:].rearrange("p (b h v) -> p b h v", b=2, h=6),
                        AO.add)
            # ---------- L3 (col-split for PE concurrency) ----------
            a3 = ap3.tile([128, 4, 16, 6, 16], BF, tag="a3")
            for g in range(4):
                for bp in range(4):
                    for ch in range(2):
                        ps3 = psAp.tile([128, 384], F, tag="psA", name="ps3")
                        for t in range(3):
                            for hf in range(2):
                                cp = 64 * hf
                                nc.tensor.matmul(
                                    ps3[cp:cp + 64, :],
                                    w3t[64 * ch:64 * ch + 64, t, cp:cp + 64],
                                    a2[64 * ch:64 * ch + 64, g,
                                       bp * 2:bp * 2 + 2, :, t:t + 32],
                                    start=(t == 0), stop=(t == 2),
                                    tile_position=(64 * ch, cp))
                        pv = ps3[:].rearrange("p (b h v e) -> p b h v e",
                                              b=2, h=6, e=2)
                        pl3 = stg.tile([128, 2, 6, 16], F, tag="l3pl")
                        nc.vector.tensor_reduce(pl3[:], pv[:],
                                                mybir.AxisListType.X, AO.max)
                        sE3 = stg.tile([128, 2, 6, 16], BF, tag="l3se")
                        nc.scalar.activation(sE3[:], pl3[:], AF.Sign,
                                             bias=vt["B3P"][:])
                        sO3 = stg.tile([128, 2, 6, 16], BF, tag="l3so")
                        nc.scalar.activation(sO3[:], pl3[:], AF.Sign,
                                             bias=vt["B3M"][:])
                        nc.vector.tensor_tensor(
                            a3[:, g, ch * 8 + bp * 2:ch * 8 + bp * 2 + 2, :, :],
                            sE3[:], sO3[:], AO.add)
            # ---------- L4 ----------
            a4 = ap4.tile([128, 4, 16, 16], BF, tag="a4")
            for ck in range(2):
                ps4 = ps4p.tile([128, 512], F, tag="ps4")
                for h in range(6):
                    nc.tensor.matmul(ps4[:], w4t[:, h, :],
                                     a3[:, ck * 2:ck * 2 + 2, :, h, :],
                                     start=(h == 0), stop=(h == 5))
                sE4 = stg.tile([128, 512], BF, tag="l4se")
                nc.scalar.activation(sE4[:], ps4[:], AF.Sign,
                                     bias=vt["B4P"][:])
                sO4 = stg.tile([128, 512], BF, tag="l4so")
                nc.scalar.activation(sO4[:], ps4[:], AF.Sign,
                                     bias=vt["B4M"][:])
                nc.vector.tensor_tensor(
                    a4[:, ck * 2:ck * 2 + 2, :, :],
                    sE4[:].rearrange("p (g b v) -> p g b v", g=2, b=16),
                    sO4[:].rearrange("p (g b v) -> p g b v", g=2, b=16),
                    AO.add)
            # ---------- FC ----------
            psf = ps4p.tile([16, 64], F, tag="psf")
            for w in range(16):
                nc.tensor.matmul(psf[0:10, :], wft[:, w, :], a4[:, :, :, w],
                                 start=(w == 0), stop=(w == 15))
            fo = stg.tile([16, 64], F, tag="fo")
            nc.scalar.activation(fo[0:10, :], psf[0:10, :], AF.Identity,
                                 bias=tbft[0:10, :], scale=1.0)
            nc.sync.dma_start(
                OUT[bt * BT:(bt + 1) * BT, :].rearrange("b o -> o b"),
                fo[0:10, :])

    nc.finalize()
    return nc


_CACHED = {}


def kernel(**inputs):
    from concourse.bass_utils import run_bass_kernel_spmd

    consts, X1s = _build_host_tensors(inputs)
    if "nc" not in _CACHED:
        _CACHED["nc"] = _build_program()
    nc = _CACHED["nc"]

    in_maps = []
    for cr in range(N_CORES):
        m = {k: np.ascontiguousarray(v) for k, v in consts.items()}
        m["X1"] = np.ascontiguousarray(X1s[cr])
        in_maps.append(m)

    res = run_bass_kernel_spmd(nc, in_maps, list(range(N_CORES)))
    out = np.concatenate([res.results[cr]["OUT"] for cr in range(N_CORES)], 0)
    return out.astype(np.float32)
